# revision 12
# baseline (speedup 1.0000x reference)
"""Trainium2 Bass kernel for nn_Matrix_Decomposition_2D (NMF multiplicative
updates), batch-parallel across 8 NeuronCores (one batch element per core).

Per-core computation (D=512, N=4096, R=64):
  xf = x.reshape(D, N)
  coef = softmax(100 * xf^T @ bases)            # init
  7x MU steps:
    coef  *= (xf^T bases) / (coef (bases^T bases) + eps)
    bases *= (xf coef)   / (bases (coef^T coef) + eps)
  coef *= ... (one extra coef update)
  out = bases @ coef^T

Precision strategy (validated numerically vs the fp32 reference):
  - init matmul (feeds the sharp softmax) in float32r (~13-14 mantissa bits,
    full PE rate at free-dim >=256); softmax math in fp32
  - everything else bf16 matmul inputs + fp32 PSUM accumulate + fp32
    elementwise masters for coef/bases
Expected absmax/scale vs fp32 reference ~9e-3.

Elementwise/copy work is batched into multi-bank PSUM group tiles
([128,16,64] num/den halves etc.) to amortize per-op engine overheads.
"""

import os

import numpy as np

import concourse.bacc as bacc
import concourse.bass as bass
import concourse.mybir as mybir
import concourse.tile as tile
from concourse.bass import ts
from concourse.bass_utils import run_bass_kernel_spmd
from concourse.masks import make_identity

F32 = mybir.dt.float32
F32R = mybir.dt.float32r
BF16 = mybir.dt.bfloat16
AX = mybir.AxisListType.X
AF = mybir.ActivationFunctionType

B = 8
D, N, R = 512, 4096, 64
KD, KN = 4, 32          # 128-row chunks of d and n
STEPS = int(os.environ.get("KERNEL_STEPS", "7"))
INV_T = 100.0
EPS = 1e-6

_CACHE = {}


def _emit(tc, nc, x_ap, b_ap, o_ap):
    # ---------------- persistent pools ----------------
    const = tc.alloc_tile_pool(name="const", bufs=1)
    xbf = tc.alloc_tile_pool(name="xbf", bufs=1)
    state = tc.alloc_tile_pool(name="state", bufs=1)
    scr = tc.alloc_tile_pool(name="scr", bufs=1)

    ident_bf = const.tile([128, 128], BF16)
    make_identity(nc, ident_bf)
    ident_f32 = const.tile([64, 64], F32)
    make_identity(nc, ident_f32)
    eps_b = const.tile([128, 1], F32)
    nc.vector.memset(eps_b, EPS)

    xf_bf = xbf.tile([128, KD, N], BF16)
    xfT_bf = xbf.tile([128, KN, 512], BF16)

    bases_f32 = state.tile([128, KD, R], F32)
    bases_bf = state.tile([128, KD, R], BF16)
    basesT_bf = state.tile([64, D], BF16)
    coef_f32 = state.tile([128, KN, R], F32)
    coef_bf = state.tile([128, KN, R], BF16)
    coefT_bf = state.tile([64, N], BF16)
    gram_b_sb = state.tile([64, R], BF16)
    gram_c_sb = state.tile([64, R], BF16)
    nb_sb = state.tile([128, KD, R], F32)

    # ---------------- setup + f32r init ----------------
    initsb = tc.alloc_tile_pool(name="initsb", bufs=1)
    stage = tc.alloc_tile_pool(name="stage", bufs=2)
    bases_r = initsb.tile([128, KD, R], F32R)
    numT0_sb = initsb.tile([64, N], F32)

    # Phase A: stream x; f32r init matmul accumulates into 8 open PSUM
    # groups (4 tiles x [64, 2, 512] = 8 banks) across the whole stream.
    psA = tc.alloc_tile_pool(name="initpsA", bufs=1, space="PSUM")

    nc.sync.dma_start(bases_f32, b_ap.rearrange("(c p) r -> p c r", p=128))
    nc.vector.tensor_copy(out=bases_bf, in_=bases_f32)
    nc.vector.tensor_copy(out=bases_r, in_=bases_f32)

    ibs = [psA.tile([64, 2, 512], F32, tag=f"initb{i}", bufs=1, name=f"ib{i}")
           for i in range(4)]
    for h in range(2 * KD):
        kd, half = h // 2, (h % 2) * 2048
        stg = stage.tile([128, 2048], F32, tag="xstage")
        nc.sync.dma_start(stg, x_ap[ts(kd, 128), half:half + 2048])
        nc.scalar.copy(out=xf_bf[:, kd, half:half + 2048], in_=stg)
        xr = stage.tile([128, 2048], F32R, tag="xr")
        nc.vector.tensor_copy(out=xr, in_=stg)
        for j in range(4):
            c = 4 * (h % 2) + j      # 512-chunk index within N
            nc.tensor.matmul(ibs[c // 2][:, c % 2, :],
                             lhsT=bases_r[:, kd, :],
                             rhs=xr[:, ts(j, 512)],
                             start=(kd == 0), stop=(kd == KD - 1),
                             skip_group_check=True)
    for i in range(4):
        nc.vector.tensor_copy(out=numT0_sb[:, ts(i, 1024)], in_=ibs[i])
    psA.release()
    stage.release()

    # Phase B: transposes (bases + xfT)
    psB = tc.alloc_tile_pool(name="initpsB", bufs=2, space="PSUM")
    btr = psB.tile([64, KD, 128], BF16, tag="xtr")
    for kd in range(KD):
        nc.tensor.matmul(btr[:, kd, :], bases_bf[:, kd, :], ident_bf, is_transpose=True, skip_group_check=True)
    nc.vector.tensor_copy(out=basesT_bf, in_=btr)
    for kn in range(KN):
        xtr = psB.tile([128, KD, 128], BF16, tag="xtr")
        for kd in range(KD):
            nc.tensor.matmul(xtr[:, kd, :], xf_bf[:, kd, ts(kn, 128)],
                             ident_bf, is_transpose=True,
                             skip_group_check=True)
        nc.vector.tensor_copy(out=xfT_bf[:, kn, :], in_=xtr)
    psB.release()

    # ---------------- softmax init (fp32), groups of 8 n-tiles ----------
    ps2 = tc.alloc_tile_pool(name="initps2", bufs=2, space="PSUM")
    for g in range(KN // 8):
        ftr = ps2.tile([128, 8, R], F32, tag="ftr")
        for j in range(8):
            nc.tensor.matmul(ftr[:, j, :], numT0_sb[:, ts(8 * g + j, 128)],
                             ident_f32, is_transpose=True,
                             skip_group_check=True)
        rmax = scr.tile([128, 8, 1], F32, tag="rmax")
        nc.vector.reduce_max(out=rmax, in_=ftr, axis=AX)
        z8 = scr.tile([128, 8, R], F32, tag="z8")
        nc.vector.tensor_sub(z8, ftr, rmax.to_broadcast([128, 8, R]))
        e8 = scr.tile([128, 8, R], F32, tag="e8")
        nc.scalar.activation(out=e8, in_=z8, func=AF.Exp, scale=INV_T)
        rsum = scr.tile([128, 8, 1], F32, tag="rsum")
        nc.vector.reduce_sum(out=rsum, in_=e8, axis=AX)
        rinv = scr.tile([128, 8, 1], F32, tag="rinv")
        nc.vector.reciprocal_approx_fast(out=rinv, in_=rsum)
        nc.vector.tensor_mul(coef_f32[:, ts(g, 8), :], e8,
                             rinv.to_broadcast([128, 8, R]))
        nc.scalar.copy(out=coef_bf[:, ts(g, 8), :], in_=coef_f32[:, ts(g, 8), :])
        ctr = ps2.tile([64, 8, 128], BF16, tag="ctr")
        for j in range(8):
            nc.tensor.matmul(ctr[:, j, :], coef_bf[:, 8 * g + j, :], ident_bf, is_transpose=True, skip_group_check=True)
        nc.vector.tensor_copy(out=coefT_bf[:, ts(g, 1024)], in_=ctr)
    ps2.release()
    initsb.release()

    ps = tc.alloc_tile_pool(name="mainps", bufs=1, space="PSUM")
    # bank budget (8): num [128,16,64]f32(2bk)x2=4 shared w/ den... see tags

    # ---------------- MU steps ----------------
    def coef_update():
        gb = ps.tile([64, R], F32, tag="small", bufs=1, name="gb")
        for kd in range(KD):
            nc.tensor.matmul(gb, lhsT=bases_bf[:, kd, :], rhs=bases_bf[:, kd, :],
                             start=(kd == 0), stop=(kd == KD - 1))
        nc.scalar.copy(out=gram_b_sb, in_=gb)

        for hf in range(2):  # halves of 16 n-tiles
            num = ps.tile([128, 16, R], F32, tag="num", bufs=2)
            den = ps.tile([128, 16, R], F32, tag="den", bufs=1)
            for j in range(16):
                kn = 16 * hf + j
                for kd in range(KD):
                    nc.tensor.matmul(num[:, j, :], lhsT=xf_bf[:, kd, ts(kn, 128)],
                                     rhs=bases_bf[:, kd, :],
                                     start=(kd == 0), stop=(kd == KD - 1),
                                     skip_group_check=True)
                nc.tensor.matmul(den[:, j, :], lhsT=coefT_bf[:, ts(kn, 128)],
                                 rhs=gram_b_sb, start=True, stop=True,
                                 skip_group_check=True)
            cslice = coef_f32[:, ts(hf, 16), :]
            rcp = scr.tile([128, 16, R], F32, tag="rcp")
            nc.vector.reciprocal_approx_fast(out=rcp, in_=den)
            t = scr.tile([128, 16, R], F32, tag="t")
            nc.vector.tensor_mul(t, cslice, num)
            nc.vector.tensor_mul(cslice, t, rcp)
            nc.scalar.copy(out=coef_bf[:, ts(hf, 16), :], in_=cslice)
            for g in range(2):
                ctr = ps.tile([64, 8, 128], BF16, tag="tr8", bufs=1)
                for j in range(8):
                    kn = 16 * hf + 8 * g + j
                    nc.tensor.matmul(ctr[:, j, :], coef_bf[:, kn, :], ident_bf, is_transpose=True, skip_group_check=True)
                nc.vector.tensor_copy(
                    out=coefT_bf[:, ts(2 * hf + g, 1024)], in_=ctr)

    def bases_update():
        gc = ps.tile([64, R], F32, tag="small", bufs=1, name="gc")
        for kn in range(KN):
            nc.tensor.matmul(gc, lhsT=coef_bf[:, kn, :], rhs=coef_bf[:, kn, :],
                             start=(kn == 0), stop=(kn == KN - 1))
        nc.scalar.copy(out=gram_c_sb, in_=gc)

        nb = ps.tile([128, KD, R], F32, tag="small", bufs=1, name="nb")
        for kd in range(KD):
            for kn in range(KN):
                nc.tensor.matmul(nb[:, kd, :], lhsT=xfT_bf[:, kn, ts(kd, 128)],
                                 rhs=coef_bf[:, kn, :],
                                 start=(kn == 0), stop=(kn == KN - 1),
                                 skip_group_check=True)
        nc.vector.tensor_copy(out=nb_sb, in_=nb)

        db = ps.tile([128, KD, R], F32, tag="small", bufs=1, name="db")
        for kd in range(KD):
            nc.tensor.matmul(db[:, kd, :], lhsT=basesT_bf[:, ts(kd, 128)],
                             rhs=gram_c_sb, start=True, stop=True,
                             skip_group_check=True)
        rcp = scr.tile([128, KD, R], F32, tag="rcpb")
        nc.vector.reciprocal_approx_fast(out=rcp, in_=db)
        t = scr.tile([128, KD, R], F32, tag="tb")
        nc.vector.tensor_mul(t, bases_f32, nb_sb)
        nc.vector.tensor_mul(bases_f32, t, rcp)
        nc.scalar.copy(out=bases_bf, in_=bases_f32)
        btr = ps.tile([64, 8, 128], BF16, tag="tr8", bufs=1)
        for kd in range(KD):
            nc.tensor.matmul(btr[:, kd, :], bases_bf[:, kd, :], ident_bf, is_transpose=True, skip_group_check=True)
        nc.vector.tensor_copy(out=basesT_bf, in_=btr[:, :KD, :])

    for _ in range(STEPS):
        coef_update()
        bases_update()
    coef_update()

    ps.release()

    # ---------------- out = bases @ coef^T ----------------
    ops_ = tc.alloc_tile_pool(name="outps", bufs=3, space="PSUM")
    ostage = tc.alloc_tile_pool(name="ostage", bufs=2)
    for kd in range(KD):
        o_sb = ostage.tile([128, N], F32, tag="ostage")
        for c in range(4):
            op = ops_.tile([128, 2, 512], F32, tag="ob")
            for j in range(2):
                nc.tensor.matmul(op[:, j, :], lhsT=basesT_bf[:, ts(kd, 128)],
                                 rhs=coefT_bf[:, ts(2 * c + j, 512)],
                                 start=True, stop=True, skip_group_check=True)
            if c % 2 == 0:
                nc.vector.tensor_copy(out=o_sb[:, ts(c, 1024)], in_=op)
            else:
                nc.scalar.copy(out=o_sb[:, ts(c, 1024)], in_=op)
        nc.sync.dma_start(o_ap[ts(kd, 128), :], o_sb)

    ostage.release()
    ops_.release()
    scr.release()
    state.release()
    xbf.release()
    const.release()


def build_program():
    if "nc" in _CACHE:
        return _CACHE["nc"]
    nc = bacc.Bacc("TRN2", target_bir_lowering=False, debug=False)
    x_ap = nc.dram_tensor("x", [D, N], F32, kind="ExternalInput").ap()
    b_ap = nc.dram_tensor("bases", [D, R], F32, kind="ExternalInput").ap()
    o_ap = nc.dram_tensor("out", [D, N], F32, kind="ExternalOutput").ap()
    with tile.TileContext(nc) as tc:
        _emit(tc, nc, x_ap, b_ap, o_ap)
    nc.compile()
    _CACHE["nc"] = nc
    return nc


LAST_EXEC_NS = None


def kernel(x: np.ndarray, bases: np.ndarray) -> np.ndarray:
    global LAST_EXEC_NS
    assert x.shape == (B, D, 64, 64) and bases.shape == (B, D, R)
    nc = build_program()
    in_maps = [
        {"x": np.ascontiguousarray(x[b].reshape(D, N), dtype=np.float32),
         "bases": np.ascontiguousarray(bases[b], dtype=np.float32)}
        for b in range(B)
    ]
    trace = bool(int(os.environ.get("KERNEL_TRACE", "0")))
    res = run_bass_kernel_spmd(nc, in_maps, core_ids=list(range(B)), trace=trace)
    LAST_EXEC_NS = res.exec_time_ns
    out = np.stack([res.results[b]["out"] for b in range(B)])
    return out.reshape(B, D, 64, 64).astype(np.float32)
